# revision 16
# baseline (speedup 1.0000x reference)
"""CRF Viterbi decode (torchcrf semantics) on 8 Trainium2 NeuronCores.

Strategy: pure data parallel over batch (1024 rows -> 128 rows/core, one row
per SBUF partition).

Forward: ONE fused custom-DVE instruction per step computes, for every
segment j in [0,32):

    w[b,j,i] = (h_{t-1}[b,i] + T[i,j]) + em_t[b,j]     (ref association)
    h_t[b,j]  = max_i w[b,j,i]

The operand stream is [P, 32 segments, 33]: slot 0 of segment j carries
em_t[b,j] (scattered into place by the otherwise-idle Activation engine,
off the critical path), slots 1..32 carry T[:,j] against a broadcast
stride-1 read of the previous step's scores straight out of `hist`.
The op is a hand-built 3-uop COUNT-cycling FSM (assembled with
dve_spec's documented state machinery): an em-state (1 element)
re-latches the segment's em into the swap flop (exactly what the stock
latch-init state does) and re-seeds the MAX scan to -FLT_MAX, then a
body state (32 elements) runs the fused add-add-maxscan.  The output
access pattern is [32 segments @ +1, 33 elements @ -1] so each
segment's final (maximal) element lands at hist[1 + t*32 + j] while
every running-prefix write falls on an address that a later element of
this instruction or the next step's h overwrites — the single write
stream deposits h_t densely into `hist` with no extra copy.  Verified
bit-exact on the device against the numpy recurrence.

Backward (recomputes each backpointer instead of storing them):
    onehot(tag_{s+1}) -> 32x32-block vector-transpose -> 4 diagonal
    tile_position matmuls gather transsel[b,i] = trans[i, tag_{s+1}(b)];
    tmp = (hist_s + transsel) + em_{s+1}[tag]  (associations match the
    ref exactly) with its row-max fused in one custom-DVE instruction
    (MAX accumulator), then max_index (first-index ties = jnp.argmax).

Inputs are taken at full shape; sharding/gather happens on host inside
kernel().  All candidate arithmetic is bit-exact vs the jax reference.
"""

import dataclasses
import sys

import numpy as np

if "/opt/trn_rl_repo" not in sys.path:
    sys.path.insert(0, "/opt/trn_rl_repo")

B, T, K = 1024, 1024, 32
NCORES = 8
BL = B // NCORES  # 128 batch rows per core
TC = 64  # time chunk for em streaming
SEGN = K + 1  # 33: [em_t[j], T[0,j], ..., T[31,j]] per segment
NSTREAM = K * SEGN  # 1056 elements per forward step
NEG_BIG = -3.0e38
POS_BIG = 3.0e38

FWD_OP_NAME = "VITERBI_FWD_STEP_ANT"
BWD_CAND_OP_NAME = "LN_BWD_DX_MAX_ANT"


def _register_bwd_cand_op():
    """(in0 - in1*s0 - s1)*imm2 with a fused MAX accumulator — the backward's
    candidate combine and its row-max in one stock-lowerable instruction."""
    import concourse.dve_ops as dve_ops
    from concourse.dve_spec import Spec, Src0, Src1, C0, C1, C2, AluOp, lower
    from concourse.dve_uop import DveOpSpec

    name = BWD_CAND_OP_NAME
    if name in dve_ops._SUB_OPCODE_FOR_NAME:
        return next(op for op in dve_ops.OPS if op.name == name)

    def _ref(in0, in1, s0, s1, imm2):
        o = (in0.astype(np.float32) - in1 * s0 - s1) * imm2
        return o, o.max(axis=-1, keepdims=True)

    spec = Spec(
        body=(Src0 - Src1 * C0 - C1) * C2,
        accum=AluOp.MAX,
        reference=_ref,
    )
    row = dve_ops._CUSTOM_DVE_ROW_BASE + len(dve_ops.OPS)
    assert row < 0x20
    shas = {}
    for ver in ("v3", "v4"):
        uops = lower(spec, ver=ver)
        shas[ver] = DveOpSpec(name=name, opcode=row, uops=uops,
                              rd1_en=True).sha(ver)
    op = dve_ops.DveOp(name, spec, subdim=False, uops_sha=shas)
    dve_ops.OPS.append(op)
    dve_ops._SUB_OPCODE_FOR_NAME[name] = row
    dve_ops.CUSTOM_DVE_SPECS[name] = spec
    return op


def _ref_viterbi_fwd(in0, in1, c0, c1, c2):
    """Faithful grid semantics of the fused step: in0 [P,S,33] broadcast
    scores ([0, h_prev]), in1 [P,S,33] ttbe (em at slot 0).  Element k of
    segment j carries the running max of w[:,j,1:k] (the em slot carries
    the -FLT_MAX scan reseed)."""
    a = np.asarray(in0, np.float32)
    b = np.asarray(in1, np.float32)
    v = a[:, :, 1:] + b[:, :, 1:]
    w = v + b[:, :, 0:1]
    m = np.maximum.accumulate(w, axis=2)
    out = np.full(b.shape, np.float32(-3.4028235e38), np.float32)
    out[:, :, 1:] = m
    return out


def _register_fwd_op():
    """Register the fused forward op: body scan(MAX,(Src0+Src1)+Latch(Src1))
    with a custom COUNT-cycling FSM (em-latch + scan-reseed state per
    segment)."""
    import concourse.dve_ops as dve_ops
    import concourse.dve_spec as DS
    from concourse.dve_uop import DveOpSpec, N_LANES, N_STAGES, Trigger

    name = FWD_OP_NAME
    if name in dve_ops._SUB_OPCODE_FOR_NAME:
        return next(op for op in dve_ops.OPS if op.name == name)

    def make_spec():
        return DS.Spec(
            body=DS.scan(DS.AluOp.MAX, (DS.Src0 + DS.Src1) + DS.Latch(DS.Src1)),
            reference=_ref_viterbi_fwd,
        )

    def build_uops(ver):
        spec = make_spec()
        DS._validate_body(spec, ver)
        sp = DS._hoist_stream_invariant_ops(spec)
        scans = DS._collect(sp.body, DS.Scan)
        latches = DS._collect(sp.body, DS.Latch)
        p = DS._build_placement(sp, scans, N_STAGES[ver], N_LANES[ver])
        seed_ov, _ = DS._scan_overrides(scans, p.node_stage)
        latch = latches[0]
        latch_ov = DS._latch_init_stages(latch.expr, p.latch_read_stage(latch))
        assert not (set(seed_ov) & set(latch_ov))
        em_ov = {**seed_ov, **latch_ov}
        Tg = Trigger
        s_em_first = DS._State(
            placement=p, overrides=em_ov,
            trigger=(Tg.SRC_TENSOR_DONE, Tg.COUNT, Tg.NONE), next=(0, 1, 0),
            repeat=1, consume=(True, True), write_out=True,
        )
        s_body = DS._State(
            placement=p, consume=(True, True),
            trigger=(Tg.SRC_TENSOR_DONE, Tg.COUNT, Tg.NONE), next=(0, 2, 0),
            repeat=K,
        )
        s_em_loop = dataclasses.replace(s_em_first, next=(0, 1, 0))
        return [DS._assemble(s) for s in (s_em_first, s_body, s_em_loop)]

    row = dve_ops._CUSTOM_DVE_ROW_BASE + len(dve_ops.OPS)
    assert row < 0x20, "custom-DVE opcode rows exhausted"

    @dataclasses.dataclass(frozen=True)
    class _UopDveOp(dve_ops.DveOp):
        def compile(self, ver):
            key = (self.name, ver)
            if (r := dve_ops._COMPILE_CACHE.get(key)) is not None:
                return r
            result = DveOpSpec(name=self.name, opcode=row,
                               uops=build_uops(ver), rd1_en=True)
            dve_ops._COMPILE_CACHE[key] = result
            return result

    op = _UopDveOp(name, make_spec(), subdim=False, uops_sha={})
    dve_ops.OPS.append(op)
    dve_ops._SUB_OPCODE_FOR_NAME[name] = row
    dve_ops.CUSTOM_DVE_SPECS[name] = op.spec
    return op


def _emit_fwd_step(nc, op, in0, in1, out):
    import concourse.bass as bass_mod
    from concourse import bass_isa, mybir
    from concourse.dve_ops import get_dve_sub_opcode

    v = nc.vector
    m = v.bass.m
    if op.name not in m.ant_custom_dve_ops:
        m.ant_custom_dve_ops = sorted({*m.ant_custom_dve_ops, op.name})
    op.compile(bass_mod.dve_ver_for(v.bass.trn_type))
    shape = bass_isa.CustomDveShape.STT  # 2-free-dim src1
    isa_opcode = v.bass.isa.Opcode[
        f"NEURON_ISA_TPB_OPCODE_CUSTOM_DVE_ANT_{shape.slot()}"
    ].value
    zero = lambda: mybir.ImmediateValue(dtype=mybir.dt.float32, value=0.0)
    return v.add_instruction(
        bass_isa.InstCustomDveAnt(
            name=v.bass.get_next_instruction_name(),
            op_name=op.name, rd1_en=True, subdim=0, imm2=0.0,
            shape=shape, row=get_dve_sub_opcode(op.name),
            isa_opcode=isa_opcode,
            ins=[
                v.lower_ap(in0, for_isa=True, opt=False),
                v.lower_ap(in1, for_isa=True, opt=False),
                zero(), zero(),
            ],
            outs=[v.lower_ap(out, for_isa=True, opt=False)],
        )
    )


def build_nc(t_steps: int = T, tc: int = TC):
    """Build + compile the per-core Bass program (same NEFF on all 8 cores)."""
    import concourse.tile as tile
    from concourse import bacc, mybir

    fwd_op = _register_fwd_op()
    bwd_cand_op = _register_bwd_cand_op()

    f32 = mybir.dt.float32
    u32 = mybir.dt.uint32
    i32 = mybir.dt.int32
    Alu = mybir.AluOpType
    Ax = mybir.AxisListType

    nsteps = t_steps
    nchunks = (nsteps + tc - 1) // tc
    assert nsteps % tc == 0

    nc = bacc.Bacc(
        "TRN2", target_bir_lowering=False, debug=False, enable_asserts=False
    )

    em_d = nc.dram_tensor("em", [BL, nsteps * K], f32, kind="ExternalInput").ap()
    ttbe_d = nc.dram_tensor("ttbe", [BL, NSTREAM], f32, kind="ExternalInput").ap()
    tmov_d = nc.dram_tensor("tmov", [128, K], f32, kind="ExternalInput").ap()
    endt_d = nc.dram_tensor("endt", [BL, K], f32, kind="ExternalInput").ap()
    iota_d = nc.dram_tensor("iota", [BL, K], u32, kind="ExternalInput").ap()
    tags_d = nc.dram_tensor("tags", [BL, nsteps], i32, kind="ExternalOutput").ap()

    with tile.TileContext(nc) as tc_ctx:
        _body(nc, tc_ctx, mybir, Alu, Ax, f32, u32, i32,
              em_d, ttbe_d, tmov_d, endt_d, iota_d, tags_d,
              nsteps, tc, nchunks, fwd_op, bwd_cand_op)

    nc.compile()
    return nc


def _body(nc, tc_ctx, mybir, Alu, Ax, f32, u32, i32,
          em_d, ttbe_d, tmov_d, endt_d, iota_d, tags_d,
          nsteps, tc, nchunks, fwd_op, bwd_cand_op):
    from contextlib import ExitStack

    from concourse.ap import AP

    ctx = ExitStack()
    with ctx:
        const_pool = ctx.enter_context(tc_ctx.tile_pool(name="const", bufs=1))
        hist_pool = ctx.enter_context(tc_ctx.tile_pool(name="hist", bufs=1))
        em_pool = ctx.enter_context(tc_ctx.tile_pool(name="em", bufs=2))
        work_pool = ctx.enter_context(tc_ctx.tile_pool(name="work", bufs=1))
        tags8_pool = ctx.enter_context(tc_ctx.tile_pool(name="tags8", bufs=2))
        psum_pool = ctx.enter_context(
            tc_ctx.tile_pool(name="psum", bufs=2, space="PSUM")
        )

        # ---- constants ----
        tbs = [const_pool.tile([BL, NSTREAM], f32, name=f"ttbe{i}")
               for i in range(2)]
        for tb in tbs:
            nc.sync.dma_start(tb[:], ttbe_d[:])
        tmov = const_pool.tile([128, K], f32)  # trans.T tiled x4 (PE weights)
        nc.sync.dma_start(tmov[:], tmov_d[:])
        endt = const_pool.tile([BL, K], f32)
        nc.sync.dma_start(endt[:], endt_d[:])
        iota = const_pool.tile([BL, K], u32)
        nc.sync.dma_start(iota[:], iota_d[:])

        # ---- working tiles ----
        # hist[1 + t*K + j] = h_t[j].  The fused op's out AP is
        # [32 segments @ +1, 33 elements @ -1] from base 1 + t*K + 32: each
        # segment's last element (the segment max) lands at 1 + t*K + j, and
        # every other (running-prefix) write falls on an address that a later
        # element of this op or the next op's h overwrites — the out stream
        # deposits h densely into hist with no extra copy.  1 front pad
        # (step 1's score operand reads one junk slot) + K tail spill.
        hist = hist_pool.tile([BL, 1 + nsteps * K + K], f32)
        m8 = work_pool.tile([BL, 8], f32)
        u_t = work_pool.tile([BL, K], f32)
        tmp = work_pool.tile([BL, K], f32)
        emsel = work_pool.tile([BL, 1], f32)
        onehot = work_pool.tile([BL, K], f32)
        vt = work_pool.tile([BL, K], f32)
        tagout = work_pool.tile([BL, nsteps], i32)

        nc.vector.memset(m8[:], POS_BIG)

        tb3s = [tb[:].rearrange("p (j e) -> p j e", e=SEGN) for tb in tbs]

        def out_ap(t):
            b0 = hist[:, 1 + t * K + K : 1 + t * K + K + 1]
            return AP(b0.tensor, b0.offset, [list(b0.ap[0]), [1, K], [-1, SEGN]])

        def score_ap(t):
            # [junk, h_{t-1}[0..31]] broadcast over segments; the junk slot
            # pairs with the em element, whose Src0 value is never used.
            return (hist[:, (t - 1) * K : (t - 1) * K + SEGN]
                    [:, None, :].broadcast_to([BL, K, SEGN]))

        # ================= forward =================
        # em chunk DMAs are issued one chunk ahead so a chunk's first
        # scatter never waits on its own DMA (+sem propagation).
        emfs = {}

        def fetch_fwd(c):
            if c < nchunks and c not in emfs:
                emfs[c] = em_pool.tile([BL, tc * K], f32, tag="emchunk",
                                       name=f"emf{c}")
                nc.sync.dma_start(
                    emfs[c][:], em_d[:, c * tc * K : (c + 1) * tc * K]
                )

        fetch_fwd(0)
        for c in range(nchunks):
            fetch_fwd(c + 1)
            emf = emfs.pop(c)
            for tloc in range(tc):
                t = c * tc + tloc
                em_sl = emf[:, tloc * K : (tloc + 1) * K]
                if t == 0:
                    # h_0 = start + em[0] (start folded on host)
                    nc.vector.tensor_copy(hist[:, 1 : 1 + K], em_sl)
                    continue
                tb3 = tb3s[t % 2]
                # em_t[j] -> segment-j slot 0, on the Activation engine
                nc.scalar.copy(tb3[:, :, 0:1], em_sl[:, :, None])
                _emit_fwd_step(nc, fwd_op, score_ap(t), tb3, out_ap(t))

        # ================= final argmax =================
        # ref: score = h[T-1] + end_transitions, then argmax (first index)
        tags8_cur = tags8_pool.tile([BL, tc * 8], u32, tag="t8")
        nc.vector.tensor_tensor(
            tmp[:], hist[:, 1 + (nsteps - 1) * K : 1 + nsteps * K], endt[:],
            Alu.add
        )
        nc.vector.max(m8[:], tmp[:])
        last_slot = (nsteps - 1) - (nchunks - 1) * tc
        nc.vector.max_index(
            tags8_cur[:, last_slot * 8 : last_slot * 8 + 8], m8[:], tmp[:]
        )

        # ================= backward =================
        # recompute backpointers step by step (bit-exact vs ref)
        from concourse.dve_ops import TENSOR_TENSOR_REDUCE as _CTTR

        tags8_by_chunk = {nchunks - 1: tags8_cur}
        # em[s+1] chunk DMAs, prefetched one chunk ahead (descending order)
        embws = {}

        def fetch_bwd(c):
            if c >= 0 and c not in embws:
                n_em = tc if c < nchunks - 1 else tc - 1
                embws[c] = em_pool.tile([BL, tc * K], f32, tag="emchunk",
                                        name=f"embw{c}")
                nc.sync.dma_start(
                    embws[c][:, : n_em * K],
                    em_d[:, (c * tc + 1) * K : (c * tc + 1 + n_em) * K],
                )

        fetch_bwd(nchunks - 1)
        for c in range(nchunks - 1, -1, -1):
            fetch_bwd(c - 1)
            embw = embws.pop(c)
            if c not in tags8_by_chunk:
                tags8_by_chunk[c] = tags8_pool.tile(
                    [BL, tc * 8], u32, tag="t8", name=f"t8c{c}"
                )
            t8c = tags8_by_chunk[c]

            s_hi = min(nsteps - 2, (c + 1) * tc - 1)
            for s in range(s_hi, c * tc - 1, -1):
                tloc = s - c * tc
                sp1 = s + 1
                cp1 = sp1 // tc
                t8p = tags8_by_chunk[cp1]
                slot = sp1 - cp1 * tc
                nc.vector.tensor_tensor(
                    onehot[:],
                    iota[:],
                    t8p[:, slot * 8 : slot * 8 + 1].broadcast_to([BL, K]),
                    Alu.is_equal,
                )
                nc.vector.transpose(vt[:], onehot[:])
                # transsel[b,i] = trans[i, tag_b] via 4 diagonal 32x32 matmuls
                tsel = psum_pool.tile([BL, K], f32, tag="tsel")
                for r in range(4):
                    nc.tensor.matmul(
                        tsel[32 * r : 32 * r + 32, :],
                        vt[32 * r : 32 * r + 32, :],
                        tmov[32 * r : 32 * r + 32, :],
                        start=True,
                        stop=True,
                        tile_position=(32 * r, 32 * r),
                    )
                # emselneg[b] = -em_{s+1}[b, tag_{s+1}(b)] (exact gather)
                nc.vector._custom_dve(
                    _CTTR,
                    out=u_t[:],
                    in0=onehot[:],
                    in1=embw[:, tloc * K : (tloc + 1) * K],
                    s0=0.0,
                    s1=-1.0,
                    accum_out=emsel[:],
                )
                # tmp = (hist_s - tsel*(-1) - (-emsel))*1 = (hist_s+tsel)+emsel
                # fused with its row-max (accum) in one instruction
                nc.vector._custom_dve(
                    bwd_cand_op,
                    out=tmp[:],
                    in0=hist[:, 1 + s * K : 1 + (s + 1) * K],
                    in1=tsel[:],
                    s0=-1.0,
                    s1=emsel[:],
                    imm2=1.0,
                    accum_out=m8[:, 0:1],
                )
                nc.vector.max_index(
                    t8c[:, tloc * 8 : tloc * 8 + 8], m8[:], tmp[:]
                )

            # compact this chunk's tags (slot stride 8 -> dense) on ScalarE
            t83 = t8c[:].rearrange("p (s e) -> p s e", e=8)
            nc.scalar.copy(
                tagout[:, c * tc : (c + 1) * tc][:, :, None], t83[:, :, 0:1]
            )
            nc.sync.dma_start(
                tags_d[:, c * tc : (c + 1) * tc], tagout[:, c * tc : (c + 1) * tc]
            )
            if c + 1 in tags8_by_chunk:
                del tags8_by_chunk[c + 1]


_NC_CACHE = {}


def _get_nc(t_steps=T, tc=TC):
    key = (t_steps, tc)
    if key not in _NC_CACHE:
        _NC_CACHE[key] = build_nc(t_steps, tc)
    return _NC_CACHE[key]


def make_in_maps(inputs, start_transitions, end_transitions, transitions,
                 t_steps=T):
    """Host-side shard + constant prep. Returns list of per-core input dicts."""
    inputs = np.asarray(inputs, np.float32)
    start = np.asarray(start_transitions, np.float32)
    end = np.asarray(end_transitions, np.float32)
    trans = np.asarray(transitions, np.float32)

    ttbe_row = np.zeros((K, SEGN), np.float32)
    ttbe_row[:, 1:] = trans.T  # segment j, slots 1..32 = T[:, j]
    ttbe = np.ascontiguousarray(
        np.broadcast_to(ttbe_row.reshape(1, NSTREAM), (BL, NSTREAM))
    )
    tmov = np.ascontiguousarray(np.tile(trans.T, (4, 1)))
    endt = np.ascontiguousarray(np.broadcast_to(end.reshape(1, K), (BL, K)))
    iota = np.ascontiguousarray(
        np.broadcast_to(np.arange(K, dtype=np.uint32), (BL, K))
    )

    in_maps = []
    for ci in range(NCORES):
        em = np.array(
            inputs[ci * BL : (ci + 1) * BL, :t_steps].reshape(BL, t_steps * K)
        )
        # fold start_transitions into em[0] (same association as the ref)
        em[:, :K] = start.reshape(1, K) + em[:, :K]
        in_maps.append(
            {"em": em, "ttbe": ttbe, "tmov": tmov, "endt": endt, "iota": iota}
        )
    return in_maps


_last_result = None


def kernel(inputs, mask, start_transitions, end_transitions, transitions):
    global _last_result
    mask = np.asarray(mask)
    if not mask.all():
        return _numpy_fallback(
            np.asarray(inputs, np.float32), mask,
            np.asarray(start_transitions, np.float32),
            np.asarray(end_transitions, np.float32),
            np.asarray(transitions, np.float32),
        )

    from concourse.bass_utils import run_bass_kernel_spmd

    nc = _get_nc()
    in_maps = make_in_maps(inputs, start_transitions, end_transitions, transitions)
    res = run_bass_kernel_spmd(nc, in_maps, core_ids=list(range(NCORES)))
    _last_result = res
    tags = np.concatenate([res.results[i]["tags"] for i in range(NCORES)], axis=0)
    return tags.astype(np.int32)


def _numpy_fallback(inputs, mask, start, end, trans):
    """Vectorized numpy Viterbi matching torchcrf/ref semantics (general mask)."""
    em = np.swapaxes(inputs, 0, 1)  # [T, B, K]
    mk = np.swapaxes(mask, 0, 1)  # [T, B]
    nT, nB, nK = em.shape
    score = start[None, :] + em[0]
    hist = np.zeros((nT - 1, nB, nK), np.int32)
    for t in range(1, nT):
        cand = score[:, :, None] + trans[None, :, :] + em[t][:, None, :]
        bp = np.argmax(cand, axis=1).astype(np.int32)
        ns = np.max(cand, axis=1)
        m = mk[t][:, None]
        score = np.where(m, ns, score)
        hist[t - 1] = bp
    score = score + end[None, :]
    tag = np.argmax(score, axis=1).astype(np.int32)
    tags = np.zeros((nT, nB), np.int32)
    tags[nT - 1] = tag
    for t in range(nT - 2, -1, -1):
        prev = np.take_along_axis(hist[t], tag[:, None], axis=1)[:, 0]
        prev = np.where(mk[t + 1], prev, tag)
        tags[t] = prev
        tag = prev
    return np.swapaxes(tags, 0, 1).astype(np.int32)


# revision 20
# speedup vs baseline: 1.0804x; 1.0804x over previous
"""CRF Viterbi decode (torchcrf semantics) on 8 Trainium2 NeuronCores.

Strategy: pure data parallel over batch (1024 rows -> 128 rows/core, one row
per SBUF partition).

Forward: ONE fused custom-DVE instruction per step computes, for every
segment j in [0,32):

    w[b,j,i] = (h_{t-1}[b,i] + T[i,j]) + em_t[b,j]     (ref association)
    h_t[b,j]  = max_i w[b,j,i]

The operand stream is [P, 32 segments, 33]: slot 0 of segment j carries
em_t[b,j] (scattered into place by the otherwise-idle Activation engine,
off the critical path), slots 1..32 carry T[:,j] against a broadcast
stride-1 read of the previous step's scores straight out of `hist`.
The op is a hand-built 3-uop COUNT-cycling FSM (assembled with
dve_spec's documented state machinery): an em-state (1 element)
re-latches the segment's em into the swap flop (exactly what the stock
latch-init state does) and re-seeds the MAX scan to -FLT_MAX, then a
body state (32 elements) runs the fused add-add-maxscan.  The output
access pattern is [32 segments @ +1, 33 elements @ -1] so each
segment's final (maximal) element lands at hist[1 + t*32 + j] while
every running-prefix write falls on an address that a later element of
this instruction or the next step's h overwrites — the single write
stream deposits h_t densely into `hist` with no extra copy.  Verified
bit-exact on the device against the numpy recurrence.

Backward (recomputes each backpointer instead of storing them):
    onehot(tag_{s+1}) -> 32x32-block vector-transpose -> 4 diagonal
    tile_position matmuls gather transsel[b,i] = trans[i, tag_{s+1}(b)];
    tmp = (hist_s + transsel) + em_{s+1}[tag]  (associations match the
    ref exactly) with its row-max fused in one custom-DVE instruction
    (MAX accumulator), then max_index (first-index ties = jnp.argmax).

Inputs are taken at full shape; sharding/gather happens on host inside
kernel().  All candidate arithmetic is bit-exact vs the jax reference.
"""

import dataclasses
import sys

import numpy as np

if "/opt/trn_rl_repo" not in sys.path:
    sys.path.insert(0, "/opt/trn_rl_repo")

B, T, K = 1024, 1024, 32
NCORES = 8
BL = B // NCORES  # 128 batch rows per core
TC = 64  # time chunk for em streaming
SEGN = K + 1  # 33: [em_t[j], T[0,j], ..., T[31,j]] per segment
NSTREAM = K * SEGN  # 1056 elements per forward step
NEG_BIG = -3.0e38
POS_BIG = 3.0e38

FWD_OP_NAME = "VITERBI_FWD_STEP_ANT"
BWD_CAND_OP_NAME = "LN_BWD_DX_MAX_ANT"


def _register_bwd_cand_op():
    """(in0 - in1*s0 - s1)*imm2 with a fused MAX accumulator — the backward's
    candidate combine and its row-max in one stock-lowerable instruction."""
    import concourse.dve_ops as dve_ops
    from concourse.dve_spec import Spec, Src0, Src1, C0, C1, C2, AluOp, lower
    from concourse.dve_uop import DveOpSpec

    name = BWD_CAND_OP_NAME
    if name in dve_ops._SUB_OPCODE_FOR_NAME:
        return next(op for op in dve_ops.OPS if op.name == name)

    def _ref(in0, in1, s0, s1, imm2):
        o = (in0.astype(np.float32) - in1 * s0 - s1) * imm2
        return o, o.max(axis=-1, keepdims=True)

    spec = Spec(
        body=(Src0 - Src1 * C0 - C1) * C2,
        accum=AluOp.MAX,
        reference=_ref,
    )
    row = dve_ops._CUSTOM_DVE_ROW_BASE + len(dve_ops.OPS)
    assert row < 0x20
    shas = {}
    for ver in ("v3", "v4"):
        uops = lower(spec, ver=ver)
        shas[ver] = DveOpSpec(name=name, opcode=row, uops=uops,
                              rd1_en=True).sha(ver)
    op = dve_ops.DveOp(name, spec, subdim=False, uops_sha=shas)
    dve_ops.OPS.append(op)
    dve_ops._SUB_OPCODE_FOR_NAME[name] = row
    dve_ops.CUSTOM_DVE_SPECS[name] = spec
    return op


def _register_bwd_vt_op():
    """Transposed onehot straight from the tag via the StreamTranspose read
    path: out[32r+j, b] = (tag[32r+b] == j).  in0 = tag broadcast [P,32]
    read through TransposeMode.TRANSPOSE; s0 = per-partition lane index
    (p mod 32).  (The numpy reference mirrors the post-transpose semantics;
    execution uses the uop table.)  Verified on device by probe8.py."""
    import concourse.dve_ops as dve_ops
    import concourse.dve_spec as DS
    from concourse.dve_uop import (
        DveOpSpec, N_LANES, N_STAGES, OpConfig, TransposeMode,
    )

    name = "VT_ONEHOT_ANT"
    if name in dve_ops._SUB_OPCODE_FOR_NAME:
        return next(op for op in dve_ops.OPS if op.name == name)

    spec = DS.Spec(
        body=DS.eq(DS.Src0, DS.C0),
        reference=lambda in0, in1, c0, c1, c2: (
            np.asarray(in0, np.float32) == np.asarray(c0, np.float32)
        ).astype(np.float32),
    )

    def build_uops(ver):
        DS._validate_body(spec, ver)
        sp = DS._hoist_stream_invariant_ops(spec)
        scans = DS._collect(sp.body, DS.Scan)
        p = DS._build_placement(sp, scans, N_STAGES[ver], N_LANES[ver])
        return [DS._assemble(DS._State(placement=p, consume=(True, False)))]

    row = dve_ops._CUSTOM_DVE_ROW_BASE + len(dve_ops.OPS)
    assert row < 0x20

    @dataclasses.dataclass(frozen=True)
    class _Op(dve_ops.DveOp):
        def compile(self, ver):
            key = (self.name, ver)
            if (r := dve_ops._COMPILE_CACHE.get(key)) is not None:
                return r
            result = DveOpSpec(
                name=self.name, opcode=row, uops=build_uops(ver),
                rd1_en=False,
                op=OpConfig(transpose_mode=TransposeMode.TRANSPOSE),
            )
            dve_ops._COMPILE_CACHE[key] = result
            return result

    op = _Op(name, spec, subdim=False, uops_sha={})
    dve_ops.OPS.append(op)
    dve_ops._SUB_OPCODE_FOR_NAME[name] = row
    dve_ops.CUSTOM_DVE_SPECS[name] = spec
    return op


def _register_em_gather_op():
    """accum_out[b] = -em[b, tag_b]: tag rides as a (broadcast) u32 stream
    (scalar slots must be f32), em as the second stream."""
    import concourse.dve_ops as dve_ops
    from concourse.dve_spec import (
        Spec, Src0, Src1, Zero, Idx, eq, AluOp, lower,
    )
    from concourse.dve_uop import DveOpSpec

    name = "EM_GATHER_NEG_ANT"
    if name in dve_ops._SUB_OPCODE_FOR_NAME:
        return next(op for op in dve_ops.OPS if op.name == name)

    def _ref(in0, in1, s0, s1, imm2):
        p, n = in0.shape
        idx = np.arange(n, dtype=np.float32)[None, :]
        out = (idx == in0.astype(np.float32)).astype(np.float32) * (
            0.0 - in1.astype(np.float32))
        return out, out.sum(axis=1, keepdims=True)

    spec = Spec(body=eq(Idx, Src0) * (Zero - Src1), accum=AluOp.ADD,
                reference=_ref)
    row = dve_ops._CUSTOM_DVE_ROW_BASE + len(dve_ops.OPS)
    assert row < 0x20
    shas = {}
    for ver in ("v3", "v4"):
        uops = lower(spec, ver=ver)
        shas[ver] = DveOpSpec(name=name, opcode=row, uops=uops,
                              rd1_en=True).sha(ver)
    op = dve_ops.DveOp(name, spec, subdim=False, uops_sha=shas)
    dve_ops.OPS.append(op)
    dve_ops._SUB_OPCODE_FOR_NAME[name] = row
    dve_ops.CUSTOM_DVE_SPECS[name] = spec
    return op


def _emit_vt(nc, op, tag_bcast, iotap, out):
    import concourse.bass as bass_mod
    from concourse import bass_isa, mybir
    from concourse.dve_ops import get_dve_sub_opcode

    v = nc.vector
    m = v.bass.m
    if op.name not in m.ant_custom_dve_ops:
        m.ant_custom_dve_ops = sorted({*m.ant_custom_dve_ops, op.name})
    op.compile(bass_mod.dve_ver_for(v.bass.trn_type))
    shape = bass_isa.CustomDveShape.TTSS
    isa_opcode = v.bass.isa.Opcode[
        f"NEURON_ISA_TPB_OPCODE_CUSTOM_DVE_ANT_{shape.slot()}"
    ].value
    imm = mybir.ImmediateValue(dtype=mybir.dt.float32, value=0.0)
    return v.add_instruction(
        bass_isa.InstCustomDveAnt(
            name=v.bass.get_next_instruction_name(),
            op_name=op.name, rd1_en=False, subdim=0, imm2=0.0,
            shape=shape, row=get_dve_sub_opcode(op.name),
            isa_opcode=isa_opcode,
            ins=[v.lower_ap(tag_bcast, for_isa=True, opt=False),
                 v.lower_ap(iotap, for_isa=True), imm],
            outs=[v.lower_ap(out, for_isa=True, opt=False)],
        )
    )


def _ref_viterbi_fwd(in0, in1, c0, c1, c2):
    """Faithful grid semantics of the fused step: in0 [P,S,33] broadcast
    scores ([0, h_prev]), in1 [P,S,33] ttbe (em at slot 0).  Element k of
    segment j carries the running max of w[:,j,1:k] (the em slot carries
    the -FLT_MAX scan reseed)."""
    a = np.asarray(in0, np.float32)
    b = np.asarray(in1, np.float32)
    v = a[:, :, 1:] + b[:, :, 1:]
    w = v + b[:, :, 0:1]
    m = np.maximum.accumulate(w, axis=2)
    out = np.full(b.shape, np.float32(-3.4028235e38), np.float32)
    out[:, :, 1:] = m
    return out


def _register_fwd_op():
    """Register the fused forward op: body scan(MAX,(Src0+Src1)+Latch(Src1))
    with a custom COUNT-cycling FSM (em-latch + scan-reseed state per
    segment)."""
    import concourse.dve_ops as dve_ops
    import concourse.dve_spec as DS
    from concourse.dve_uop import DveOpSpec, N_LANES, N_STAGES, Trigger

    name = FWD_OP_NAME
    if name in dve_ops._SUB_OPCODE_FOR_NAME:
        return next(op for op in dve_ops.OPS if op.name == name)

    def make_spec():
        return DS.Spec(
            body=DS.scan(DS.AluOp.MAX, (DS.Src0 + DS.Src1) + DS.Latch(DS.Src1)),
            reference=_ref_viterbi_fwd,
        )

    def build_uops(ver):
        spec = make_spec()
        DS._validate_body(spec, ver)
        sp = DS._hoist_stream_invariant_ops(spec)
        scans = DS._collect(sp.body, DS.Scan)
        latches = DS._collect(sp.body, DS.Latch)
        p = DS._build_placement(sp, scans, N_STAGES[ver], N_LANES[ver])
        seed_ov, _ = DS._scan_overrides(scans, p.node_stage)
        latch = latches[0]
        latch_ov = DS._latch_init_stages(latch.expr, p.latch_read_stage(latch))
        assert not (set(seed_ov) & set(latch_ov))
        em_ov = {**seed_ov, **latch_ov}
        Tg = Trigger
        s_em_first = DS._State(
            placement=p, overrides=em_ov,
            trigger=(Tg.SRC_TENSOR_DONE, Tg.COUNT, Tg.NONE), next=(0, 1, 0),
            repeat=1, consume=(True, True), write_out=True,
        )
        s_body = DS._State(
            placement=p, consume=(True, True),
            trigger=(Tg.SRC_TENSOR_DONE, Tg.COUNT, Tg.NONE), next=(0, 2, 0),
            repeat=K,
        )
        s_em_loop = dataclasses.replace(s_em_first, next=(0, 1, 0))
        return [DS._assemble(s) for s in (s_em_first, s_body, s_em_loop)]

    row = dve_ops._CUSTOM_DVE_ROW_BASE + len(dve_ops.OPS)
    assert row < 0x20, "custom-DVE opcode rows exhausted"

    @dataclasses.dataclass(frozen=True)
    class _UopDveOp(dve_ops.DveOp):
        def compile(self, ver):
            key = (self.name, ver)
            if (r := dve_ops._COMPILE_CACHE.get(key)) is not None:
                return r
            result = DveOpSpec(name=self.name, opcode=row,
                               uops=build_uops(ver), rd1_en=True)
            dve_ops._COMPILE_CACHE[key] = result
            return result

    op = _UopDveOp(name, make_spec(), subdim=False, uops_sha={})
    dve_ops.OPS.append(op)
    dve_ops._SUB_OPCODE_FOR_NAME[name] = row
    dve_ops.CUSTOM_DVE_SPECS[name] = op.spec
    return op


def _emit_fwd_step(nc, op, in0, in1, out):
    import concourse.bass as bass_mod
    from concourse import bass_isa, mybir
    from concourse.dve_ops import get_dve_sub_opcode

    v = nc.vector
    m = v.bass.m
    if op.name not in m.ant_custom_dve_ops:
        m.ant_custom_dve_ops = sorted({*m.ant_custom_dve_ops, op.name})
    op.compile(bass_mod.dve_ver_for(v.bass.trn_type))
    shape = bass_isa.CustomDveShape.STT  # 2-free-dim src1
    isa_opcode = v.bass.isa.Opcode[
        f"NEURON_ISA_TPB_OPCODE_CUSTOM_DVE_ANT_{shape.slot()}"
    ].value
    zero = lambda: mybir.ImmediateValue(dtype=mybir.dt.float32, value=0.0)
    return v.add_instruction(
        bass_isa.InstCustomDveAnt(
            name=v.bass.get_next_instruction_name(),
            op_name=op.name, rd1_en=True, subdim=0, imm2=0.0,
            shape=shape, row=get_dve_sub_opcode(op.name),
            isa_opcode=isa_opcode,
            ins=[
                v.lower_ap(in0, for_isa=True, opt=False),
                v.lower_ap(in1, for_isa=True, opt=False),
                zero(), zero(),
            ],
            outs=[v.lower_ap(out, for_isa=True, opt=False)],
        )
    )


def build_nc(t_steps: int = T, tc: int = TC):
    """Build + compile the per-core Bass program (same NEFF on all 8 cores)."""
    import concourse.tile as tile
    from concourse import bacc, mybir

    fwd_op = _register_fwd_op()
    bwd_cand_op = _register_bwd_cand_op()
    vt_op = _register_bwd_vt_op()
    emg_op = _register_em_gather_op()

    f32 = mybir.dt.float32
    u32 = mybir.dt.uint32
    i32 = mybir.dt.int32
    Alu = mybir.AluOpType
    Ax = mybir.AxisListType

    nsteps = t_steps
    nchunks = (nsteps + tc - 1) // tc
    assert nsteps % tc == 0

    nc = bacc.Bacc(
        "TRN2", target_bir_lowering=False, debug=False, enable_asserts=False
    )

    em_d = nc.dram_tensor("em", [BL, nsteps * K], f32, kind="ExternalInput").ap()
    ttbe_d = nc.dram_tensor("ttbe", [BL, NSTREAM], f32, kind="ExternalInput").ap()
    tmov_d = nc.dram_tensor("tmov", [128, K], f32, kind="ExternalInput").ap()
    endt_d = nc.dram_tensor("endt", [BL, K], f32, kind="ExternalInput").ap()
    iota_d = nc.dram_tensor("iota", [BL, K], u32, kind="ExternalInput").ap()
    iotap_d = nc.dram_tensor("iotap", [BL, 1], f32, kind="ExternalInput").ap()
    tags_d = nc.dram_tensor("tags", [BL, nsteps], i32, kind="ExternalOutput").ap()

    with tile.TileContext(nc) as tc_ctx:
        _body(nc, tc_ctx, mybir, Alu, Ax, f32, u32, i32,
              em_d, ttbe_d, tmov_d, endt_d, iota_d, iotap_d, tags_d,
              nsteps, tc, nchunks, fwd_op, bwd_cand_op, vt_op, emg_op)

    nc.compile()
    return nc


def _body(nc, tc_ctx, mybir, Alu, Ax, f32, u32, i32,
          em_d, ttbe_d, tmov_d, endt_d, iota_d, iotap_d, tags_d,
          nsteps, tc, nchunks, fwd_op, bwd_cand_op, vt_op, emg_op):
    from contextlib import ExitStack

    from concourse.ap import AP

    ctx = ExitStack()
    with ctx:
        const_pool = ctx.enter_context(tc_ctx.tile_pool(name="const", bufs=1))
        hist_pool = ctx.enter_context(tc_ctx.tile_pool(name="hist", bufs=1))
        em_pool = ctx.enter_context(tc_ctx.tile_pool(name="em", bufs=2))
        work_pool = ctx.enter_context(tc_ctx.tile_pool(name="work", bufs=1))
        tags8_pool = ctx.enter_context(tc_ctx.tile_pool(name="tags8", bufs=2))
        psum_pool = ctx.enter_context(
            tc_ctx.tile_pool(name="psum", bufs=2, space="PSUM")
        )

        # ---- constants ----
        tbs = [const_pool.tile([BL, NSTREAM], f32, name=f"ttbe{i}")
               for i in range(2)]
        for tb in tbs:
            nc.sync.dma_start(tb[:], ttbe_d[:])
        tmov = const_pool.tile([128, K], f32)  # trans.T tiled x4 (PE weights)
        nc.sync.dma_start(tmov[:], tmov_d[:])
        endt = const_pool.tile([BL, K], f32)
        nc.sync.dma_start(endt[:], endt_d[:])
        iota = const_pool.tile([BL, K], u32)
        nc.sync.dma_start(iota[:], iota_d[:])
        iotap = const_pool.tile([BL, 1], f32)
        nc.sync.dma_start(iotap[:], iotap_d[:])

        # ---- working tiles ----
        # hist[1 + t*K + j] = h_t[j].  The fused op's out AP is
        # [32 segments @ +1, 33 elements @ -1] from base 1 + t*K + 32: each
        # segment's last element (the segment max) lands at 1 + t*K + j, and
        # every other (running-prefix) write falls on an address that a later
        # element of this op or the next op's h overwrites — the out stream
        # deposits h densely into hist with no extra copy.  1 front pad
        # (step 1's score operand reads one junk slot) + K tail spill.
        hist = hist_pool.tile([BL, 1 + nsteps * K + K], f32)
        m8 = work_pool.tile([BL, 8], f32)
        u_t = work_pool.tile([BL, K], f32)
        tmp = work_pool.tile([BL, K], f32)
        emsel = work_pool.tile([BL, 1], f32)
        onehot = work_pool.tile([BL, K], f32)
        vt = work_pool.tile([BL, K], f32)
        tagout = work_pool.tile([BL, nsteps], i32)

        nc.vector.memset(m8[:], POS_BIG)

        tb3s = [tb[:].rearrange("p (j e) -> p j e", e=SEGN) for tb in tbs]

        def out_ap(t):
            b0 = hist[:, 1 + t * K + K : 1 + t * K + K + 1]
            return AP(b0.tensor, b0.offset, [list(b0.ap[0]), [1, K], [-1, SEGN]])

        def score_ap(t):
            # [junk, h_{t-1}[0..31]] broadcast over segments; the junk slot
            # pairs with the em element, whose Src0 value is never used.
            return (hist[:, (t - 1) * K : (t - 1) * K + SEGN]
                    [:, None, :].broadcast_to([BL, K, SEGN]))

        # ================= forward =================
        # em chunk DMAs are issued one chunk ahead so a chunk's first
        # scatter never waits on its own DMA (+sem propagation).
        emfs = {}

        def fetch_fwd(c):
            if c < nchunks and c not in emfs:
                emfs[c] = em_pool.tile([BL, tc * K], f32, tag="emchunk",
                                       name=f"emf{c}")
                nc.sync.dma_start(
                    emfs[c][:], em_d[:, c * tc * K : (c + 1) * tc * K]
                )

        fetch_fwd(0)
        for c in range(nchunks):
            fetch_fwd(c + 1)
            emf = emfs.pop(c)
            for tloc in range(tc):
                t = c * tc + tloc
                em_sl = emf[:, tloc * K : (tloc + 1) * K]
                if t == 0:
                    # h_0 = start + em[0] (start folded on host)
                    nc.vector.tensor_copy(hist[:, 1 : 1 + K], em_sl)
                    continue
                tb3 = tb3s[t % 2]
                # em_t[j] -> segment-j slot 0, on the Activation engine
                nc.scalar.copy(tb3[:, :, 0:1], em_sl[:, :, None])
                _emit_fwd_step(nc, fwd_op, score_ap(t), tb3, out_ap(t))

        # ================= final argmax =================
        # ref: score = h[T-1] + end_transitions, then argmax (first index)
        tags8_cur = tags8_pool.tile([BL, tc * 8], u32, tag="t8")
        nc.vector.tensor_tensor(
            tmp[:], hist[:, 1 + (nsteps - 1) * K : 1 + nsteps * K], endt[:],
            Alu.add
        )
        nc.vector.max(m8[:], tmp[:])
        last_slot = (nsteps - 1) - (nchunks - 1) * tc
        nc.vector.max_index(
            tags8_cur[:, last_slot * 8 : last_slot * 8 + 8], m8[:], tmp[:]
        )

        # ================= backward =================
        # recompute backpointers step by step (bit-exact vs ref)
        from concourse.dve_ops import TENSOR_TENSOR_REDUCE as _CTTR

        tags8_by_chunk = {nchunks - 1: tags8_cur}
        # em[s+1] chunk DMAs, prefetched one chunk ahead (descending order)
        embws = {}

        def fetch_bwd(c):
            if c >= 0 and c not in embws:
                n_em = tc if c < nchunks - 1 else tc - 1
                embws[c] = em_pool.tile([BL, tc * K], f32, tag="emchunk",
                                        name=f"embw{c}")
                nc.sync.dma_start(
                    embws[c][:, : n_em * K],
                    em_d[:, (c * tc + 1) * K : (c * tc + 1 + n_em) * K],
                )

        fetch_bwd(nchunks - 1)
        for c in range(nchunks - 1, -1, -1):
            fetch_bwd(c - 1)
            embw = embws.pop(c)
            if c not in tags8_by_chunk:
                tags8_by_chunk[c] = tags8_pool.tile(
                    [BL, tc * 8], u32, tag="t8", name=f"t8c{c}"
                )
            t8c = tags8_by_chunk[c]

            s_hi = min(nsteps - 2, (c + 1) * tc - 1)
            for s in range(s_hi, c * tc - 1, -1):
                tloc = s - c * tc
                sp1 = s + 1
                cp1 = sp1 // tc
                t8p = tags8_by_chunk[cp1]
                slot = sp1 - cp1 * tc
                # vt[32r+j, b] = (tag[32r+b] == j) in ONE transpose-mode op
                _emit_vt(
                    nc, vt_op,
                    t8p[:, slot * 8 : slot * 8 + 1].broadcast_to([BL, K]),
                    iotap[:, 0:1], vt[:],
                )
                # transsel[b,i] = trans[i, tag_b] via 4 diagonal 32x32 matmuls
                tsel = psum_pool.tile([BL, K], f32, tag="tsel")
                for r in range(4):
                    nc.tensor.matmul(
                        tsel[32 * r : 32 * r + 32, :],
                        vt[32 * r : 32 * r + 32, :],
                        tmov[32 * r : 32 * r + 32, :],
                        start=True,
                        stop=True,
                        tile_position=(32 * r, 32 * r),
                    )
                # emselneg[b] = -em_{s+1}[b, tag_{s+1}(b)] (exact gather)
                nc.vector._custom_dve(
                    emg_op,
                    out=u_t[:],
                    in0=t8p[:, slot * 8 : slot * 8 + 1].broadcast_to([BL, K]),
                    in1=embw[:, tloc * K : (tloc + 1) * K],
                    s0=0.0,
                    s1=0.0,
                    accum_out=emsel[:],
                )
                # tmp = (hist_s - tsel*(-1) - (-emsel))*1 = (hist_s+tsel)+emsel
                # fused with its row-max (accum) in one instruction
                nc.vector._custom_dve(
                    bwd_cand_op,
                    out=tmp[:],
                    in0=hist[:, 1 + s * K : 1 + (s + 1) * K],
                    in1=tsel[:],
                    s0=-1.0,
                    s1=emsel[:],
                    imm2=1.0,
                    accum_out=m8[:, 0:1],
                )
                nc.vector.max_index(
                    t8c[:, tloc * 8 : tloc * 8 + 8], m8[:], tmp[:]
                )

            # compact this chunk's tags (slot stride 8 -> dense) on ScalarE
            t83 = t8c[:].rearrange("p (s e) -> p s e", e=8)
            nc.scalar.copy(
                tagout[:, c * tc : (c + 1) * tc][:, :, None], t83[:, :, 0:1]
            )
            nc.sync.dma_start(
                tags_d[:, c * tc : (c + 1) * tc], tagout[:, c * tc : (c + 1) * tc]
            )
            if c + 1 in tags8_by_chunk:
                del tags8_by_chunk[c + 1]


_NC_CACHE = {}


def _get_nc(t_steps=T, tc=TC):
    key = (t_steps, tc)
    if key not in _NC_CACHE:
        _NC_CACHE[key] = build_nc(t_steps, tc)
    return _NC_CACHE[key]


def make_in_maps(inputs, start_transitions, end_transitions, transitions,
                 t_steps=T):
    """Host-side shard + constant prep. Returns list of per-core input dicts."""
    inputs = np.asarray(inputs, np.float32)
    start = np.asarray(start_transitions, np.float32)
    end = np.asarray(end_transitions, np.float32)
    trans = np.asarray(transitions, np.float32)

    ttbe_row = np.zeros((K, SEGN), np.float32)
    ttbe_row[:, 1:] = trans.T  # segment j, slots 1..32 = T[:, j]
    ttbe = np.ascontiguousarray(
        np.broadcast_to(ttbe_row.reshape(1, NSTREAM), (BL, NSTREAM))
    )
    tmov = np.ascontiguousarray(np.tile(trans.T, (4, 1)))
    endt = np.ascontiguousarray(np.broadcast_to(end.reshape(1, K), (BL, K)))
    iota = np.ascontiguousarray(
        np.broadcast_to(np.arange(K, dtype=np.uint32), (BL, K))
    )
    iotap = (np.arange(BL, dtype=np.float32) % K).reshape(BL, 1)

    in_maps = []
    for ci in range(NCORES):
        em = np.array(
            inputs[ci * BL : (ci + 1) * BL, :t_steps].reshape(BL, t_steps * K)
        )
        # fold start_transitions into em[0] (same association as the ref)
        em[:, :K] = start.reshape(1, K) + em[:, :K]
        in_maps.append(
            {"em": em, "ttbe": ttbe, "tmov": tmov, "endt": endt,
             "iota": iota, "iotap": iotap}
        )
    return in_maps


_last_result = None


def kernel(inputs, mask, start_transitions, end_transitions, transitions):
    global _last_result
    mask = np.asarray(mask)
    if not mask.all():
        return _numpy_fallback(
            np.asarray(inputs, np.float32), mask,
            np.asarray(start_transitions, np.float32),
            np.asarray(end_transitions, np.float32),
            np.asarray(transitions, np.float32),
        )

    from concourse.bass_utils import run_bass_kernel_spmd

    nc = _get_nc()
    in_maps = make_in_maps(inputs, start_transitions, end_transitions, transitions)
    res = run_bass_kernel_spmd(nc, in_maps, core_ids=list(range(NCORES)))
    _last_result = res
    tags = np.concatenate([res.results[i]["tags"] for i in range(NCORES)], axis=0)
    return tags.astype(np.int32)


def _numpy_fallback(inputs, mask, start, end, trans):
    """Vectorized numpy Viterbi matching torchcrf/ref semantics (general mask)."""
    em = np.swapaxes(inputs, 0, 1)  # [T, B, K]
    mk = np.swapaxes(mask, 0, 1)  # [T, B]
    nT, nB, nK = em.shape
    score = start[None, :] + em[0]
    hist = np.zeros((nT - 1, nB, nK), np.int32)
    for t in range(1, nT):
        cand = score[:, :, None] + trans[None, :, :] + em[t][:, None, :]
        bp = np.argmax(cand, axis=1).astype(np.int32)
        ns = np.max(cand, axis=1)
        m = mk[t][:, None]
        score = np.where(m, ns, score)
        hist[t - 1] = bp
    score = score + end[None, :]
    tag = np.argmax(score, axis=1).astype(np.int32)
    tags = np.zeros((nT, nB), np.int32)
    tags[nT - 1] = tag
    for t in range(nT - 2, -1, -1):
        prev = np.take_along_axis(hist[t], tag[:, None], axis=1)[:, 0]
        prev = np.where(mk[t + 1], prev, tag)
        tags[t] = prev
        tag = prev
    return np.swapaxes(tags, 0, 1).astype(np.int32)
